# revision 22
# baseline (speedup 1.0000x reference)
"""BertAttention (cross-attention variant) Trainium2 Bass kernel.

Strategy: data-parallel over batch (16 batches -> 8 cores x 2 batches).

Host-side prep (layout only): X^T / C^T are uploaded pre-transposed in a
partition-major [128, 6, 512] bf16 layout, weights are uploaded bf16 in
m-blocked layouts, and the q/k biases are uploaded pre-transposed
[128, 2, 6].  This removes every PE identity-transpose, the DRAM bounce
staging, and all on-device weight casts from the old design.

Per core, per batch:
  Q^T = Wq^T X^T and K^T = Wk^T C^T via PSUM-accumulated matmuls with the
  weight m-block stationary (bias added on the PSUM->SBUF eviction),
  V = C Wv in natural layout with an appended ones-column per head (the
  softmax denominator), S^T = K Q^T per head with two heads row-packed on
  the PE via tile_position (the two 64-row matmuls run concurrently),
  P = exp(S/8) on the ACT engine (no max-subtraction needed: scores are
  O(1) by construction), O[q, 65] = P^T(as lhsT) @ V_aug; the last column
  gives the denominator; normalize with reciprocal + broadcast multiply.

The schedule software-pipelines the attention pairs two deep against the
ACT exp stream and feeds all remaining projection work (b0 m>=1, all of
b1) as PE fillers between score tiles, so the PE never sits idle while
ACT catches up.  DMA is issued on three independent rings (sync / gpsimd
/ vector) in critical-path priority order.
"""

import os
import sys
from collections import deque

import numpy as np
import ml_dtypes

for _p in ("/opt/trn_rl_repo", "/root/.axon_site/_ro/trn_rl_repo"):
    if os.path.isdir(_p) and _p not in sys.path:
        sys.path.insert(0, _p)

import concourse.bass as bass  # noqa: E402
import concourse.tile as tile  # noqa: E402
from concourse import bacc, mybir  # noqa: E402
from concourse.bass_utils import run_bass_kernel_spmd  # noqa: E402

# Problem constants (hardcoded per spec)
B, S, D, H, HD = 16, 512, 768, 12, 64
NCORES = 8
BL = B // NCORES  # batches per core = 2
DT = D // 128     # 6 d-tiles
KT = S // 128     # 4 k-token tiles
QT = S // 128     # 4 q-token tiles
HP = H // 2       # 6 head pairs
P = 128

f32 = mybir.dt.float32
bf16 = mybir.dt.bfloat16
AF = mybir.ActivationFunctionType

_CACHE = {}


def _emit(tc, xt_ap, ct_ap, wqk_ap, wv_ap, bqk_ap, bv_ap, out):
    nc = tc.nc
    from contextlib import ExitStack

    with ExitStack() as ctx:
        wpool = ctx.enter_context(tc.tile_pool(name="wpool", bufs=1))
        xpool = ctx.enter_context(tc.tile_pool(name="xpool", bufs=1))
        qkpool = ctx.enter_context(tc.tile_pool(name="qkpool", bufs=1))
        vapool = ctx.enter_context(tc.tile_pool(name="vapool", bufs=1))
        expool = ctx.enter_context(tc.tile_pool(name="expool", bufs=24))
        orowp = ctx.enter_context(tc.tile_pool(name="orowp", bufs=1))
        smallp = ctx.enter_context(tc.tile_pool(name="smallp", bufs=8))
        proj_p = ctx.enter_context(tc.tile_pool(name="proj_p", bufs=2, space="PSUM"))
        st_p = ctx.enter_context(tc.tile_pool(name="st_p", bufs=2, space="PSUM"))
        pv_p = ctx.enter_context(tc.tile_pool(name="pv_p", bufs=2, space="PSUM"))

        # ---- SBUF tiles ----
        wqk_sb = wpool.tile([P, DT, 2, DT, P], bf16, name="wqk")
        wv_sb = wpool.tile([P, DT, D], bf16, name="wv")
        bqk_sb = wpool.tile([P, 2, DT], f32, name="bqk")
        bv_sb = wpool.tile([P, D], f32, name="bv")
        xt_sb = [xpool.tile([P, DT, S], bf16, name=f"xt{b}") for b in range(BL)]
        ct_sb = [xpool.tile([P, DT, S], bf16, name=f"ct{b}") for b in range(BL)]
        qt_sb = [qkpool.tile([P, DT, S], bf16, name=f"qt{b}") for b in range(BL)]
        kt_sb = [qkpool.tile([P, DT, S], bf16, name=f"kt{b}") for b in range(BL)]
        va_sb = [vapool.tile([P, KT, H, HD + 1], bf16, name=f"va{b}") for b in range(BL)]
        orow = [orowp.tile([P, QT, D], f32, name=f"orow{b}") for b in range(BL)]

        # ---- DMA issues: two HWDGE rings (sync + scalar), critical-path
        #      first.  gpsimd (SWDGE, slow) carries only the tiny bias
        #      broadcast.  sync: qk weights, X^T(b0), b1 tensors, outputs;
        #      scalar: C^T(b0), biases, V weights (issued before the ACT
        #      warmup so the issue cost hides in the DMA-wait window). ----
        def wqk_dma(m):
            nc.sync.dma_start(out=wqk_sb[:, m], in_=wqk_ap[m])

        nc.sync.dma_start(out=xt_sb[0][:, 0:3, :], in_=xt_ap[0][:, 0:3, :])
        nc.scalar.dma_start(out=wqk_sb[:, 0], in_=wqk_ap[0])
        nc.sync.dma_start(out=xt_sb[0][:, 3:6, :], in_=xt_ap[0][:, 3:6, :])
        nc.scalar.dma_start(out=bqk_sb, in_=bqk_ap)
        nc.scalar.dma_start(out=ct_sb[0][:, 0:3, :], in_=ct_ap[0][:, 0:3, :])
        nc.scalar.dma_start(out=ct_sb[0][:, 3:6, :], in_=ct_ap[0][:, 3:6, :])
        bv_row = wpool.tile([1, D], f32, name="bv_row")
        nc.scalar.dma_start(out=bv_row, in_=bv_ap.rearrange("(o d) -> o d", o=1))
        wqk_dma(1)
        nc.sync.dma_start(out=wv_sb, in_=wv_ap)
        wqk_dma(2)
        wqk_dma(3)
        wqk_dma(4)
        wqk_dma(5)
        nc.sync.dma_start(out=ct_sb[1], in_=ct_ap[1])
        nc.sync.dma_start(out=xt_sb[1], in_=xt_ap[1])
        nc.gpsimd.partition_broadcast(bv_sb, bv_row)

        # ---- ACT exp-table warmup: trigger the ~2.7us table load while the
        #      DMAs are still in flight ----
        warm = smallp.tile([P, 4], f32, name="warm")
        nc.gpsimd.memset(warm, 0.0)
        warm2 = smallp.tile([P, 4], f32, name="warm2")
        nc.scalar.activation(out=warm2, in_=warm, func=AF.Exp)

        # ---- PE clock warmup: junk matmuls ramp the PE out of its low
        #      p-state while the first input DMAs are still in flight, so
        #      real matmuls start at full clock ----
        junk = smallp.tile([P, S], bf16, name="junk")
        nc.gpsimd.memset(junk, 0.0)
        for grp in range(8):
            psj = proj_p.tile([P, S], f32, name="psj", tag="proj")
            for r in range(2):
                nc.tensor.matmul(
                    psj, lhsT=junk[:, 0:P], rhs=junk, start=(r == 0), stop=(r == 1)
                )

        # ---- projection micro-parts: each part is ~3 matmuls so fillers can
        #      be interleaved between score tiles at fine grain without ever
        #      starving the ACT exp stream or blocking on PSUM ----
        def qk_parts(b, iqk, m):
            state = {}
            src = xt_sb[b] if iqk == 0 else ct_sb[b]

            def p1():
                ps = proj_p.tile([P, S], f32, name="psproj", tag="proj")
                state["ps"] = ps
                for k in range(3):
                    nc.tensor.matmul(
                        ps,
                        lhsT=wqk_sb[:, m, iqk, k, :],
                        rhs=src[:, k, :],
                        start=(k == 0),
                        stop=False,
                    )

            def p2():
                ps = state["ps"]
                for k in range(3, DT):
                    nc.tensor.matmul(
                        ps,
                        lhsT=wqk_sb[:, m, iqk, k, :],
                        rhs=src[:, k, :],
                        start=False,
                        stop=(k == DT - 1),
                    )
                dst = qt_sb[b] if iqk == 0 else kt_sb[b]
                nc.vector.tensor_scalar_add(
                    out=dst[:, m, :], in0=ps, scalar1=bqk_sb[:, iqk, m : m + 1]
                )

            return [p1, p2]

        def v_parts(b, m):
            state = {}

            def mk_mm(key, lo, hi, krange):
                def f():
                    if key not in state:
                        state[key] = proj_p.tile([P, S], f32, name="psv", tag="proj")
                    ps = state[key]
                    for k in krange:
                        nc.tensor.matmul(
                            ps[:, 0 : hi - lo],
                            lhsT=ct_sb[b][:, k, m * P : (m + 1) * P],
                            rhs=wv_sb[:, k, lo:hi],
                            start=(k == 0),
                            stop=(k == DT - 1),
                        )
                return f

            a1 = mk_mm("A", 0, 512, range(3))
            a2m = mk_mm("A", 0, 512, range(3, DT))
            b1 = mk_mm("B", 512, 768, range(3))
            b2m = mk_mm("B", 512, 768, range(3, DT))

            def a2():
                a2m()
                nc.vector.tensor_add(
                    out=va_sb[b][:, m, 0:8, 0:HD],
                    in0=state["A"].rearrange("p (h x) -> p h x", x=HD),
                    in1=bv_sb[:, 0:512].rearrange("p (h x) -> p h x", x=HD),
                )

            def b2():
                b2m()
                nc.vector.tensor_add(
                    out=va_sb[b][:, m, 8:12, 0:HD],
                    in0=state["B"][:, 0:256].rearrange("p (h x) -> p h x", x=HD),
                    in1=bv_sb[:, 512:768].rearrange("p (h x) -> p h x", x=HD),
                )
                nc.gpsimd.memset(va_sb[b][:, m, :, HD : HD + 1], 1.0)

            return [a1, a2, b1, b2]

        # ---- filler machinery ----
        fillers = []
        marks = {}
        fidx = [0]

        def fill(n):
            for _ in range(min(n, len(fillers) - fidx[0])):
                fillers[fidx[0]]()
                fidx[0] += 1

        def fill_until(idx):
            while fidx[0] < idx:
                fillers[fidx[0]]()
                fidx[0] += 1

        # ---- attention halves ----
        def st_half(b, hp):
            if (b, hp) != (0, 0):
                fill_until(marks[("qk", b, hp)])
            exs = []
            for kt in range(KT):
                if kt >= 2:
                    fill(1)
                st = st_p.tile([P, 2, S], f32, name="st", tag="st")
                for pr in (0, 1):
                    nc.tensor.matmul(
                        st[:, pr, :],
                        lhsT=kt_sb[b][pr * 64 : (pr + 1) * 64, hp, kt * P : (kt + 1) * P],
                        rhs=qt_sb[b][pr * 64 : (pr + 1) * 64, hp, :],
                        start=True,
                        stop=True,
                        tile_position=(pr * 64, 0),
                    )
                ex = expool.tile([P, 2, S], bf16, name="ex", tag="ex")
                nc.scalar.activation(out=ex, in_=st, func=AF.Exp, scale=0.125)
                exs.append(ex)
            return exs

        def pv_half(b, hp, exs):
            fill_until(marks[("va", b)])
            for pr in (0, 1):
                h = 2 * hp + pr
                pv = pv_p.tile([P, QT, HD + 1], f32, name="pv", tag="pv")
                for q in range(QT):
                    for kt in range(KT):
                        nc.tensor.matmul(
                            pv[:, q, :],
                            lhsT=exs[kt][:, pr, q * P : (q + 1) * P],
                            rhs=va_sb[b][:, kt, h, :],
                            start=(kt == 0),
                            stop=(kt == KT - 1),
                        )
                rc = smallp.tile([P, QT], f32, name="rc", tag="rc")
                nc.vector.reciprocal(
                    rc, pv[:, :, HD : HD + 1].rearrange("p a b -> p (a b)")
                )
                rc_b = bass.AP(
                    tensor=rc.tensor,
                    offset=rc.offset,
                    ap=[list(rc.ap[0]), [1, QT], [0, HD]],
                )
                nc.vector.tensor_mul(
                    out=orow[b][:, :, h * HD : (h + 1) * HD],
                    in0=pv[:, :, 0:HD],
                    in1=rc_b,
                )

        # ---- schedule ----
        # Fillers in dependency order: qk m-parts for pair (b,hp) are marked
        # so st_half force-fills up to them; va(b) is marked for pv_half.
        # V parts are interleaved between qk chunks so the forced fill burst
        # at the first pv of each batch stays below the ACT exp backlog.
        for m in (1, 2):
            fillers.extend(qk_parts(0, 0, m))
            fillers.extend(qk_parts(0, 1, m))
            marks[("qk", 0, m)] = len(fillers)
        fillers.extend(v_parts(0, 0))
        fillers.extend(qk_parts(0, 0, 3))
        fillers.extend(v_parts(0, 1))
        fillers.extend(qk_parts(0, 1, 3))
        marks[("qk", 0, 3)] = len(fillers)
        fillers.extend(v_parts(0, 2))
        fillers.extend(v_parts(0, 3))
        marks[("va", 0)] = len(fillers)
        for m in (4, 5):
            fillers.extend(qk_parts(0, 0, m))
            fillers.extend(qk_parts(0, 1, m))
            marks[("qk", 0, m)] = len(fillers)
        for m in range(2):
            fillers.extend(v_parts(1, m))
            fillers.extend(qk_parts(1, 0, m))
            fillers.extend(qk_parts(1, 1, m))
            marks[("qk", 1, m)] = len(fillers)
        fillers.extend(v_parts(1, 2))
        fillers.extend(v_parts(1, 3))
        marks[("va", 1)] = len(fillers)
        for m in range(2, DT):
            fillers.extend(qk_parts(1, 0, m))
            fillers.extend(qk_parts(1, 1, m))
            marks[("qk", 1, m)] = len(fillers)

        for f in qk_parts(0, 0, 0) + qk_parts(0, 1, 0):
            f()

        pairs = [(0, hp) for hp in range(HP)] + [(1, hp) for hp in range(HP)]
        # Software pipeline: PV(i) consumes exps computed >=2 iterations
        # earlier so the PE never waits on the ACT exp stream; the depth
        # grows toward the end so ACT drains its queue early and the last
        # PV is not gated on a late exp.
        exps_q = [st_half(0, 0)]
        exps_q.append(st_half(0, 1))
        emitted = 2

        def depth(i):
            return 2 if i < 2 else (3 if i < 5 else (4 if i < 7 else 5))

        for i, (b, hp) in enumerate(pairs):
            while emitted < len(pairs) and emitted <= i + depth(i):
                exps_q.append(st_half(*pairs[emitted]))
                emitted += 1
            fill(3)
            pv_half(b, hp, exps_q.pop(0))
            o = out[b].rearrange("(q p) d -> p q d", p=P)
            nc.sync.dma_start(
                out=o[:, :, hp * P : (hp + 1) * P],
                in_=orow[b][:, :, hp * P : (hp + 1) * P],
            )
        # leftover projection work forms the tail: the kernel ends on plain
        # matmuls instead of the exp->pv->evict->DMA dependency chain
        fill(len(fillers))


def build_program():
    if "nc" in _CACHE:
        return _CACHE["nc"]
    nc = bacc.Bacc("TRN2", target_bir_lowering=False, debug=False)
    xt = nc.dram_tensor("xt", [BL, P, DT, S], bf16, kind="ExternalInput").ap()
    ct = nc.dram_tensor("ct", [BL, P, DT, S], bf16, kind="ExternalInput").ap()
    wqk = nc.dram_tensor("wqk", [DT, P, 2, DT, P], bf16, kind="ExternalInput").ap()
    wv = nc.dram_tensor("wv", [P, DT, D], bf16, kind="ExternalInput").ap()
    bqk = nc.dram_tensor("bqk", [P, 2, DT], f32, kind="ExternalInput").ap()
    bv = nc.dram_tensor("bv", [D], f32, kind="ExternalInput").ap()
    out = nc.dram_tensor("out", [BL, S, D], f32, kind="ExternalOutput").ap()
    with tile.TileContext(nc) as tc:
        _emit(tc, xt, ct, wqk, wv, bqk, bv, out)
    nc.compile()
    _CACHE["nc"] = nc
    return nc


def make_in_maps(hidden_states, context, Wq, bq, Wk, bk, Wv, bv):
    """Host-side sharding + layout prep (transpose / reshape / dtype cast
    only -- every FLOP of the model runs on device)."""
    bf = ml_dtypes.bfloat16
    hs = np.asarray(hidden_states, np.float32)
    ctx = np.asarray(context, np.float32)

    def tpose(x):  # [S, D] -> [128, DT, S] bf16, d = a*128 + p
        return x.T.reshape(DT, P, S).transpose(1, 0, 2).astype(bf)

    xt_all = np.ascontiguousarray(np.stack([tpose(hs[b]) for b in range(B)]))
    ct_all = np.ascontiguousarray(np.stack([tpose(ctx[b]) for b in range(B)]))

    def wblock(w):  # [D, D] -> [DT_m, 128p, DT_a, 128mc], d_in=a*128+p, d_out=m*128+mc
        return np.asarray(w, np.float32).reshape(DT, P, DT, P).transpose(2, 1, 0, 3)

    wqk = np.ascontiguousarray(
        np.stack([wblock(Wq), wblock(Wk)], axis=2).astype(bf)
    )  # [6, 128, 2, 6, 128] -- per (m, partition) a contiguous 3KB line
    wv_d = np.ascontiguousarray(
        np.asarray(Wv, np.float32).reshape(DT, P, D).transpose(1, 0, 2).astype(bf)
    )  # [128, 6, 768]
    bqk = np.ascontiguousarray(
        np.stack(
            [
                np.asarray(bq, np.float32).reshape(DT, P).T,
                np.asarray(bk, np.float32).reshape(DT, P).T,
            ],
            axis=1,
        ).astype(np.float32)
    )  # [128, 2, 6]
    bv_d = np.ascontiguousarray(np.asarray(bv, np.float32))

    common = {"wqk": wqk, "wv": wv_d, "bqk": bqk, "bv": bv_d}
    in_maps = []
    for c in range(NCORES):
        m = dict(common)
        m["xt"] = np.ascontiguousarray(xt_all[c * BL : (c + 1) * BL])
        m["ct"] = np.ascontiguousarray(ct_all[c * BL : (c + 1) * BL])
        in_maps.append(m)
    return in_maps


def run(in_maps, **kwargs):
    nc = build_program()
    return run_bass_kernel_spmd(nc, in_maps, core_ids=list(range(NCORES)), **kwargs)


def kernel(hidden_states, context, Wq, bq, Wk, bk, Wv, bv):
    in_maps = make_in_maps(hidden_states, context, Wq, bq, Wk, bk, Wv, bv)
    res = run(in_maps)
    outs = [np.asarray(res.results[i]["out"], np.float32) for i in range(NCORES)]
    return np.concatenate(outs, axis=0)


# revision 24
# speedup vs baseline: 1.0701x; 1.0701x over previous
"""BertAttention (cross-attention variant) Trainium2 Bass kernel.

Strategy: data-parallel over batch (16 batches -> 8 cores x 2 batches).

Host-side prep (layout only): X^T / C^T are uploaded pre-transposed in a
partition-major [128, 6, 512] bf16 layout, weights are uploaded bf16 in
m-blocked layouts, and the q/k biases are uploaded pre-transposed
[128, 2, 6].  This removes every PE identity-transpose, the DRAM bounce
staging, and all on-device weight casts from the old design.

Per core, per batch:
  Q^T = Wq^T X^T and K^T = Wk^T C^T via PSUM-accumulated matmuls with the
  weight m-block stationary (bias added on the PSUM->SBUF eviction),
  V = C Wv in natural layout with an appended ones-column per head (the
  softmax denominator), S^T = K Q^T per head with two heads row-packed on
  the PE via tile_position (the two 64-row matmuls run concurrently),
  P = exp(S/8) on the ACT engine (no max-subtraction needed: scores are
  O(1) by construction), O[q, 65] = P^T(as lhsT) @ V_aug; the last column
  gives the denominator; normalize with reciprocal + broadcast multiply.

The schedule software-pipelines the attention pairs two deep against the
ACT exp stream and feeds all remaining projection work (b0 m>=1, all of
b1) as PE fillers between score tiles, so the PE never sits idle while
ACT catches up.  DMA is issued on three independent rings (sync / gpsimd
/ vector) in critical-path priority order.
"""

import os
import sys
from collections import deque

import numpy as np
import ml_dtypes

for _p in ("/opt/trn_rl_repo", "/root/.axon_site/_ro/trn_rl_repo"):
    if os.path.isdir(_p) and _p not in sys.path:
        sys.path.insert(0, _p)

import concourse.bass as bass  # noqa: E402
import concourse.tile as tile  # noqa: E402
from concourse import bacc, mybir  # noqa: E402
from concourse.bass_utils import run_bass_kernel_spmd  # noqa: E402

# Problem constants (hardcoded per spec)
B, S, D, H, HD = 16, 512, 768, 12, 64
NCORES = 8
BL = B // NCORES  # batches per core = 2
DT = D // 128     # 6 d-tiles
KT = S // 128     # 4 k-token tiles
QT = S // 128     # 4 q-token tiles
HP = H // 2       # 6 head pairs
P = 128

f32 = mybir.dt.float32
bf16 = mybir.dt.bfloat16
AF = mybir.ActivationFunctionType

_CACHE = {}


def _emit(tc, xt_ap, ct_ap, wqk_ap, wv_ap, bqk_ap, bv_ap, out):
    nc = tc.nc
    from contextlib import ExitStack

    with ExitStack() as ctx:
        wpool = ctx.enter_context(tc.tile_pool(name="wpool", bufs=1))
        xpool = ctx.enter_context(tc.tile_pool(name="xpool", bufs=1))
        qkpool = ctx.enter_context(tc.tile_pool(name="qkpool", bufs=1))
        vapool = ctx.enter_context(tc.tile_pool(name="vapool", bufs=1))
        expool = ctx.enter_context(tc.tile_pool(name="expool", bufs=24))
        orowp = ctx.enter_context(tc.tile_pool(name="orowp", bufs=1))
        smallp = ctx.enter_context(tc.tile_pool(name="smallp", bufs=8))
        proj_p = ctx.enter_context(tc.tile_pool(name="proj_p", bufs=2, space="PSUM"))
        st_p = ctx.enter_context(tc.tile_pool(name="st_p", bufs=2, space="PSUM"))
        pv_p = ctx.enter_context(tc.tile_pool(name="pv_p", bufs=2, space="PSUM"))

        # ---- SBUF tiles ----
        wqk_sb = wpool.tile([P, DT, 2, DT, P], bf16, name="wqk")
        wv_sb = wpool.tile([P, DT, D], bf16, name="wv")
        bqk_sb = wpool.tile([P, 2, DT], f32, name="bqk")
        bv_sb = wpool.tile([P, D], f32, name="bv")
        xt_sb = [xpool.tile([P, DT, S], bf16, name=f"xt{b}") for b in range(BL)]
        ct_sb = [xpool.tile([P, DT, S], bf16, name=f"ct{b}") for b in range(BL)]
        qt_sb = [qkpool.tile([P, DT, S], bf16, name=f"qt{b}") for b in range(BL)]
        kt_sb = [qkpool.tile([P, DT, S], bf16, name=f"kt{b}") for b in range(BL)]
        va_sb = [vapool.tile([P, KT, H, HD + 1], bf16, name=f"va{b}") for b in range(BL)]
        orow = [orowp.tile([P, QT, D], f32, name=f"orow{b}") for b in range(BL)]

        # ---- DMA issues: two HWDGE rings (sync + scalar), critical-path
        #      first.  gpsimd (SWDGE, slow) carries only the tiny bias
        #      broadcast.  sync: qk weights, X^T(b0), b1 tensors, outputs;
        #      scalar: C^T(b0), biases, V weights (issued before the ACT
        #      warmup so the issue cost hides in the DMA-wait window). ----
        def wqk_dma(m):
            nc.sync.dma_start(out=wqk_sb[:, m], in_=wqk_ap[m])

        nc.sync.dma_start(out=xt_sb[0][:, 0:3, :], in_=xt_ap[0][:, 0:3, :])
        nc.scalar.dma_start(out=wqk_sb[:, 0], in_=wqk_ap[0])
        nc.sync.dma_start(out=xt_sb[0][:, 3:6, :], in_=xt_ap[0][:, 3:6, :])
        nc.scalar.dma_start(out=bqk_sb, in_=bqk_ap)
        nc.scalar.dma_start(out=ct_sb[0][:, 0:3, :], in_=ct_ap[0][:, 0:3, :])
        nc.sync.dma_start(out=ct_sb[0][:, 3:6, :], in_=ct_ap[0][:, 3:6, :])
        bv_row = wpool.tile([1, D], f32, name="bv_row")
        nc.scalar.dma_start(out=bv_row, in_=bv_ap.rearrange("(o d) -> o d", o=1))
        wqk_dma(1)
        nc.sync.dma_start(out=wv_sb, in_=wv_ap)
        wqk_dma(2)
        wqk_dma(3)
        wqk_dma(4)
        wqk_dma(5)
        nc.sync.dma_start(out=ct_sb[1], in_=ct_ap[1])
        nc.sync.dma_start(out=xt_sb[1], in_=xt_ap[1])
        nc.gpsimd.partition_broadcast(bv_sb, bv_row)

        # ---- ACT exp-table warmup: trigger the ~2.7us table load while the
        #      DMAs are still in flight ----
        warm = smallp.tile([P, 4], f32, name="warm")
        nc.gpsimd.memset(warm, 0.0)
        warm2 = smallp.tile([P, 4], f32, name="warm2")
        nc.scalar.activation(out=warm2, in_=warm, func=AF.Exp)

        # ---- PE clock warmup: junk matmuls ramp the PE out of its low
        #      p-state while the first input DMAs are still in flight, so
        #      real matmuls start at full clock ----
        junk = smallp.tile([P, S], bf16, name="junk")
        nc.gpsimd.memset(junk, 0.0)
        for grp in range(11):
            psj = proj_p.tile([P, S], f32, name="psj", tag="proj")
            for r in range(2):
                nc.tensor.matmul(
                    psj, lhsT=junk[:, 0:P], rhs=junk, start=(r == 0), stop=(r == 1)
                )

        # ---- projection micro-parts: each part is ~3 matmuls so fillers can
        #      be interleaved between score tiles at fine grain without ever
        #      starving the ACT exp stream or blocking on PSUM ----
        def qk_parts(b, iqk, m):
            state = {}
            src = xt_sb[b] if iqk == 0 else ct_sb[b]

            def p1():
                ps = proj_p.tile([P, S], f32, name="psproj", tag="proj")
                state["ps"] = ps
                for k in range(3):
                    nc.tensor.matmul(
                        ps,
                        lhsT=wqk_sb[:, m, iqk, k, :],
                        rhs=src[:, k, :],
                        start=(k == 0),
                        stop=False,
                    )

            def p2():
                ps = state["ps"]
                for k in range(3, DT):
                    nc.tensor.matmul(
                        ps,
                        lhsT=wqk_sb[:, m, iqk, k, :],
                        rhs=src[:, k, :],
                        start=False,
                        stop=(k == DT - 1),
                    )
                dst = qt_sb[b] if iqk == 0 else kt_sb[b]
                nc.vector.tensor_scalar_add(
                    out=dst[:, m, :], in0=ps, scalar1=bqk_sb[:, iqk, m : m + 1]
                )

            return [p1, p2]

        def v_parts(b, m):
            state = {}

            def mk_mm(key, lo, hi, krange):
                def f():
                    if key not in state:
                        state[key] = proj_p.tile([P, S], f32, name="psv", tag="proj")
                    ps = state[key]
                    for k in krange:
                        nc.tensor.matmul(
                            ps[:, 0 : hi - lo],
                            lhsT=ct_sb[b][:, k, m * P : (m + 1) * P],
                            rhs=wv_sb[:, k, lo:hi],
                            start=(k == 0),
                            stop=(k == DT - 1),
                        )
                return f

            a1 = mk_mm("A", 0, 512, range(3))
            a2m = mk_mm("A", 0, 512, range(3, DT))
            b1 = mk_mm("B", 512, 768, range(3))
            b2m = mk_mm("B", 512, 768, range(3, DT))

            def a2():
                a2m()
                nc.vector.tensor_add(
                    out=va_sb[b][:, m, 0:8, 0:HD],
                    in0=state["A"].rearrange("p (h x) -> p h x", x=HD),
                    in1=bv_sb[:, 0:512].rearrange("p (h x) -> p h x", x=HD),
                )

            def b2():
                b2m()
                nc.vector.tensor_add(
                    out=va_sb[b][:, m, 8:12, 0:HD],
                    in0=state["B"][:, 0:256].rearrange("p (h x) -> p h x", x=HD),
                    in1=bv_sb[:, 512:768].rearrange("p (h x) -> p h x", x=HD),
                )
                nc.gpsimd.memset(va_sb[b][:, m, :, HD : HD + 1], 1.0)

            return [a1, a2, b1, b2]

        # ---- filler machinery ----
        fillers = []
        marks = {}
        fidx = [0]

        def fill(n):
            for _ in range(min(n, len(fillers) - fidx[0])):
                fillers[fidx[0]]()
                fidx[0] += 1

        def fill_until(idx):
            while fidx[0] < idx:
                fillers[fidx[0]]()
                fidx[0] += 1

        # ---- attention halves ----
        def st_half(b, hp):
            if (b, hp) != (0, 0):
                fill_until(marks[("qk", b, hp)])
            exs = []
            for kt in range(KT):
                if kt >= 2:
                    fill(1)
                st = st_p.tile([P, 2, S], f32, name="st", tag="st")
                for pr in (0, 1):
                    nc.tensor.matmul(
                        st[:, pr, :],
                        lhsT=kt_sb[b][pr * 64 : (pr + 1) * 64, hp, kt * P : (kt + 1) * P],
                        rhs=qt_sb[b][pr * 64 : (pr + 1) * 64, hp, :],
                        start=True,
                        stop=True,
                        tile_position=(pr * 64, 0),
                    )
                ex = expool.tile([P, 2, S], bf16, name="ex", tag="ex")
                nc.scalar.activation(out=ex, in_=st, func=AF.Exp, scale=0.125)
                exs.append(ex)
            return exs

        def pv_half(b, hp, exs):
            fill_until(marks[("va", b)])
            for pr in (0, 1):
                h = 2 * hp + pr
                pv = pv_p.tile([P, QT, HD + 1], f32, name="pv", tag="pv")
                for q in range(QT):
                    for kt in range(KT):
                        nc.tensor.matmul(
                            pv[:, q, :],
                            lhsT=exs[kt][:, pr, q * P : (q + 1) * P],
                            rhs=va_sb[b][:, kt, h, :],
                            start=(kt == 0),
                            stop=(kt == KT - 1),
                        )
                rc = smallp.tile([P, QT], f32, name="rc", tag="rc")
                nc.vector.reciprocal(
                    rc, pv[:, :, HD : HD + 1].rearrange("p a b -> p (a b)")
                )
                rc_b = bass.AP(
                    tensor=rc.tensor,
                    offset=rc.offset,
                    ap=[list(rc.ap[0]), [1, QT], [0, HD]],
                )
                nc.vector.tensor_mul(
                    out=orow[b][:, :, h * HD : (h + 1) * HD],
                    in0=pv[:, :, 0:HD],
                    in1=rc_b,
                )

        # ---- schedule ----
        # Fillers in dependency order: qk m-parts for pair (b,hp) are marked
        # so st_half force-fills up to them; va(b) is marked for pv_half.
        # V parts are interleaved between qk chunks so the forced fill burst
        # at the first pv of each batch stays below the ACT exp backlog.
        for m in (1, 2):
            fillers.extend(qk_parts(0, 0, m))
            fillers.extend(qk_parts(0, 1, m))
            marks[("qk", 0, m)] = len(fillers)
        fillers.extend(v_parts(0, 0))
        fillers.extend(qk_parts(0, 0, 3))
        fillers.extend(v_parts(0, 1))
        fillers.extend(qk_parts(0, 1, 3))
        marks[("qk", 0, 3)] = len(fillers)
        fillers.extend(v_parts(0, 2))
        fillers.extend(v_parts(0, 3))
        marks[("va", 0)] = len(fillers)
        for m in (4, 5):
            fillers.extend(qk_parts(0, 0, m))
            fillers.extend(qk_parts(0, 1, m))
            marks[("qk", 0, m)] = len(fillers)
        for m in range(2):
            fillers.extend(v_parts(1, m))
            fillers.extend(qk_parts(1, 0, m))
            fillers.extend(qk_parts(1, 1, m))
            marks[("qk", 1, m)] = len(fillers)
        fillers.extend(v_parts(1, 2))
        fillers.extend(v_parts(1, 3))
        marks[("va", 1)] = len(fillers)
        for m in range(2, DT):
            fillers.extend(qk_parts(1, 0, m))
            fillers.extend(qk_parts(1, 1, m))
            marks[("qk", 1, m)] = len(fillers)

        for f in qk_parts(0, 0, 0) + qk_parts(0, 1, 0):
            f()

        pairs = [(0, hp) for hp in range(HP)] + [(1, hp) for hp in range(HP)]
        # Software pipeline: PV(i) consumes exps computed >=2 iterations
        # earlier so the PE never waits on the ACT exp stream; the depth
        # grows toward the end so ACT drains its queue early and the last
        # PV is not gated on a late exp.
        exps_q = [st_half(0, 0)]
        exps_q.append(st_half(0, 1))
        emitted = 2

        def depth(i):
            return 2 if i < 2 else (3 if i < 5 else (4 if i < 7 else 5))

        for i, (b, hp) in enumerate(pairs):
            while emitted < len(pairs) and emitted <= i + depth(i):
                exps_q.append(st_half(*pairs[emitted]))
                emitted += 1
            fill(3)
            pv_half(b, hp, exps_q.pop(0))
            o = out[b].rearrange("(q p) d -> p q d", p=P)
            nc.sync.dma_start(
                out=o[:, :, hp * P : (hp + 1) * P],
                in_=orow[b][:, :, hp * P : (hp + 1) * P],
            )
        # leftover projection work forms the tail: the kernel ends on plain
        # matmuls instead of the exp->pv->evict->DMA dependency chain
        fill(len(fillers))


def build_program():
    if "nc" in _CACHE:
        return _CACHE["nc"]
    nc = bacc.Bacc("TRN2", target_bir_lowering=False, debug=False)
    xt = nc.dram_tensor("xt", [BL, P, DT, S], bf16, kind="ExternalInput").ap()
    ct = nc.dram_tensor("ct", [BL, P, DT, S], bf16, kind="ExternalInput").ap()
    wqk = nc.dram_tensor("wqk", [DT, P, 2, DT, P], bf16, kind="ExternalInput").ap()
    wv = nc.dram_tensor("wv", [P, DT, D], bf16, kind="ExternalInput").ap()
    bqk = nc.dram_tensor("bqk", [P, 2, DT], f32, kind="ExternalInput").ap()
    bv = nc.dram_tensor("bv", [D], f32, kind="ExternalInput").ap()
    out = nc.dram_tensor("out", [BL, S, D], f32, kind="ExternalOutput").ap()
    with tile.TileContext(nc) as tc:
        _emit(tc, xt, ct, wqk, wv, bqk, bv, out)
    nc.compile()
    _CACHE["nc"] = nc
    return nc


def make_in_maps(hidden_states, context, Wq, bq, Wk, bk, Wv, bv):
    """Host-side sharding + layout prep (transpose / reshape / dtype cast
    only -- every FLOP of the model runs on device)."""
    bf = ml_dtypes.bfloat16
    hs = np.asarray(hidden_states, np.float32)
    ctx = np.asarray(context, np.float32)

    def tpose(x):  # [S, D] -> [128, DT, S] bf16, d = a*128 + p
        return x.T.reshape(DT, P, S).transpose(1, 0, 2).astype(bf)

    xt_all = np.ascontiguousarray(np.stack([tpose(hs[b]) for b in range(B)]))
    ct_all = np.ascontiguousarray(np.stack([tpose(ctx[b]) for b in range(B)]))

    def wblock(w):  # [D, D] -> [DT_m, 128p, DT_a, 128mc], d_in=a*128+p, d_out=m*128+mc
        return np.asarray(w, np.float32).reshape(DT, P, DT, P).transpose(2, 1, 0, 3)

    wqk = np.ascontiguousarray(
        np.stack([wblock(Wq), wblock(Wk)], axis=2).astype(bf)
    )  # [6, 128, 2, 6, 128] -- per (m, partition) a contiguous 3KB line
    wv_d = np.ascontiguousarray(
        np.asarray(Wv, np.float32).reshape(DT, P, D).transpose(1, 0, 2).astype(bf)
    )  # [128, 6, 768]
    bqk = np.ascontiguousarray(
        np.stack(
            [
                np.asarray(bq, np.float32).reshape(DT, P).T,
                np.asarray(bk, np.float32).reshape(DT, P).T,
            ],
            axis=1,
        ).astype(np.float32)
    )  # [128, 2, 6]
    bv_d = np.ascontiguousarray(np.asarray(bv, np.float32))

    common = {"wqk": wqk, "wv": wv_d, "bqk": bqk, "bv": bv_d}
    in_maps = []
    for c in range(NCORES):
        m = dict(common)
        m["xt"] = np.ascontiguousarray(xt_all[c * BL : (c + 1) * BL])
        m["ct"] = np.ascontiguousarray(ct_all[c * BL : (c + 1) * BL])
        in_maps.append(m)
    return in_maps


def run(in_maps, **kwargs):
    nc = build_program()
    return run_bass_kernel_spmd(nc, in_maps, core_ids=list(range(NCORES)), **kwargs)


def kernel(hidden_states, context, Wq, bq, Wk, bk, Wv, bv):
    in_maps = make_in_maps(hidden_states, context, Wq, bq, Wk, bk, Wv, bv)
    res = run(in_maps)
    outs = [np.asarray(res.results[i]["out"], np.float32) for i in range(NCORES)]
    return np.concatenate(outs, axis=0)
